# revision 1
# baseline (speedup 1.0000x reference)
"""Multi-head attention (B=2, S=2048, D=1024, H=16) on 8 Trainium2 NeuronCores.

Sharding: data-parallel on batch, tensor-parallel on heads.
Core c handles batch b = c // 4 and heads [4*(c%4), 4*(c%4)+4).
Each core computes its 4 heads' attention + its partial Wo projection;
the host sums the 4 partial [S, D] outputs per batch (the TP all-reduce).

Device-side layout choices (see comments inline):
- Host pre-transposes query/context to [D, S] (bf16) so every matmul
  contraction dim lands on SBUF partitions with no on-device transposes.
- Scores are computed transposed ([c, q]) so the PV matmul consumes the
  exp'd probabilities directly as the moving operand, and the softmax
  denominator comes free as a 65th row of the PV output (ones column in V).
- bf16 storage/matmul operands, fp32 PSUM accumulation throughout.
"""

import numpy as np
import ml_dtypes

import concourse.bacc as bacc
import concourse.mybir as mybir
from concourse.tile import TileContext
from concourse.bass_utils import run_bass_kernel_spmd

BF16 = mybir.dt.bfloat16
F32 = mybir.dt.float32

B, S, D, H = 2, 2048, 1024, 16
SPH = D // H          # 64
NH = 4                # heads per core
P = 128               # SBUF partitions
DC = D // P           # 8 d-chunks
CT = S // P           # 16 c-tiles
QT = S // P           # 16 q-tiles
NEG_INF = -1e9

_NC_CACHE = {}


def _build(masked: bool):
    nc = bacc.Bacc("TRN2", target_bir_lowering=False, debug=False, num_devices=8)

    qt_d = nc.declare_dram_parameter("qt", [D, S], BF16, isOutput=False)
    ct_d = nc.declare_dram_parameter("ctx", [D, S], BF16, isOutput=False)
    wq_d = nc.declare_dram_parameter("wq", [D, NH * SPH], BF16, isOutput=False)
    wk_d = nc.declare_dram_parameter("wk", [D, NH * SPH], BF16, isOutput=False)
    wv_d = nc.declare_dram_parameter("wv", [D, NH * SPH], BF16, isOutput=False)
    wo_d = nc.declare_dram_parameter("wo", [NH * SPH, D], BF16, isOutput=False)
    if masked:
        mk_d = nc.declare_dram_parameter("maskT", [S, S], BF16, isOutput=False)
    out_d = nc.declare_dram_parameter("out", [S, D], BF16, isOutput=True)

    with TileContext(nc) as tc:
        with (
            tc.tile_pool(name="const", bufs=1) as const,
            tc.tile_pool(name="work", bufs=1) as work,
            tc.tile_pool(name="pt", bufs=4) as ptp,
            tc.tile_pool(name="outp", bufs=3) as outp,
            tc.tile_pool(name="psS", bufs=2, space="PSUM") as psS,
            tc.tile_pool(name="psA", bufs=2, space="PSUM") as psA,
        ):
            # ---- stage inputs in SBUF -------------------------------------
            # weights first (small), then the 4MB inputs in 2-chunk pieces so
            # projection matmuls can start as chunks land
            wq_sb = const.tile([P, DC, NH * SPH], BF16)
            nc.sync.dma_start(out=wq_sb, in_=wq_d[:, :].rearrange("(c p) n -> p c n", p=P))
            wk_sb = const.tile([P, DC, NH * SPH], BF16)
            nc.sync.dma_start(out=wk_sb, in_=wk_d[:, :].rearrange("(c p) n -> p c n", p=P))
            wv_sb = const.tile([P, DC, NH * SPH], BF16)
            nc.sync.dma_start(out=wv_sb, in_=wv_d[:, :].rearrange("(c p) n -> p c n", p=P))
            # wo rows are (h, s); head pair t = h//2 packs two heads into the
            # partition dim (head h%2==0 -> partitions 0-63, ==1 -> 64-127).
            wo_sb = const.tile([P, 2, D], BF16)
            nc.sync.dma_start(out=wo_sb, in_=wo_d[:, :].rearrange("(t x) d -> x t d", x=P))
            qt_sb = const.tile([P, DC, S], BF16)
            qt_r = qt_d[:, :].rearrange("(c p) q -> p c q", p=P)
            ct_sb = const.tile([P, DC, S], BF16)
            ct_r = ct_d[:, :].rearrange("(c p) q -> p c q", p=P)
            for i in range(0, DC, 2):
                nc.sync.dma_start(out=ct_sb[:, i:i + 2, :], in_=ct_r[:, i:i + 2, :])
            for i in range(0, DC, 2):
                nc.sync.dma_start(out=qt_sb[:, i:i + 2, :], in_=qt_r[:, i:i + 2, :])


            # ---- projections: qT/kT [64, S] per head, packed per pair -----
            qTp = [work.tile([P, S], BF16, tag=f"qT{p}", name=f"qT{p}") for p in range(2)]
            kTp = [work.tile([P, S], BF16, tag=f"kT{p}", name=f"kT{p}") for p in range(2)]

            def emit_proj_chunk(p, which, qc4):
                src_sb = wk_sb if which == "k" else wq_sb
                x_sb = ct_sb if which == "k" else qt_sb
                dst = kTp[p] if which == "k" else qTp[p]
                ps = psS.tile([P, 512], F32, tag="S", bufs=2, name="ps")
                for dc in range(DC):
                    nc.tensor.matmul(
                        ps[:, :],
                        src_sb[:, dc, P * p:P * (p + 1)],
                        x_sb[:, dc, 512 * qc4:512 * (qc4 + 1)],
                        start=(dc == 0), stop=(dc == DC - 1),
                    )
                nc.scalar.copy(dst[:, 512 * qc4:512 * (qc4 + 1)], ps[:, :])

            def emit_proj(p):
                for which in ("k", "q"):
                    for qc4 in range(4):
                        emit_proj_chunk(p, which, qc4)

            # ---- V in natural [c, (h, s)] layout + ones column ------------
            vaug = work.tile([P, CT // 2, NH, 2, 80], BF16)
            nc.vector.memset(vaug[:, :, :, :, SPH:SPH + 1], 1.0)

            def emit_v():
                for ct in range(CT):
                    psv = psS.tile([P, NH * SPH], F32, tag="S", bufs=2, name="psv")
                    for dc in range(DC):
                        nc.tensor.matmul(
                            psv[:, :],
                            ct_sb[:, dc, P * ct:P * (ct + 1)],
                            wv_sb[:, dc, :],
                            start=(dc == 0), stop=(dc == DC - 1),
                        )
                    nc.vector.tensor_copy(
                        vaug[:, ct // 2, :, ct % 2, 0:SPH],
                        psv[:, :].rearrange("p (h s) -> p h s", h=NH),
                    )

            # ---- attention, two heads of a pair interleaved ---------------
            # Heads a=0 / a=1 of a pair live at partition bases 0 / 64, so
            # their score matmuls target different PE row groups and run
            # concurrently when emitted adjacently -- into different banks of
            # ONE S tile [128, 2, 512]. One exp op then covers both heads.
            # PSUM: S [2 banks]x2bufs + po [1 bank]x4 = 8 banks exactly.
            # outT_qc[qc]: [(a*64+s) partition, pair, 1024 q] bf16
            outT_qc = [work.tile([P, 2, 1024], BF16, tag=f"oT{qc}", name=f"oT{qc}") for qc in range(2)]

            def epilogue(po, p, a, qc4):
                # normalize rows 0-63 by reciprocal of row 64. Engines cannot
                # shift partitions: denominator row goes PSUM -> SBUF (DVE,
                # base-matched), row 64 -> row 0 via DMA, then gpsimd
                # broadcast (reads partition 0 only), recip, multiply.
                qc, off = qc4 // 2, 512 * (qc4 % 2)
                srow = outp.tile([P, 512], F32, tag="srow", name="srow")
                nc.vector.tensor_copy(srow[SPH:SPH + 1, :], po[SPH:SPH + 1, :])
                drow = outp.tile([1, 512], F32, tag="drow", name="drow")
                nc.gpsimd.dma_start(out=drow[0:1, :], in_=srow[SPH:SPH + 1, :])
                rb = outp.tile([SPH, 512], F32, tag="rb", name="rb")
                nc.gpsimd.partition_broadcast(rb, drow[0:1, :], channels=SPH)
                rb2 = outp.tile([SPH, 512], F32, tag="rb2", name="rb2")
                nc.vector.reciprocal_approx_fast(rb2, rb)
                if a == 0:
                    nc.vector.tensor_mul(outT_qc[qc][0:SPH, p, off:off + 512],
                                         po[0:SPH, :], rb2)
                else:
                    ot = ptp.tile([SPH, 512], BF16, tag="ott", name="ot")
                    nc.vector.tensor_mul(ot, po[0:SPH, :], rb2)
                    # partition shift 0-63 -> 64-127 has to go through DMA
                    nc.gpsimd.dma_start(out=outT_qc[qc][SPH:P, p, off:off + 512], in_=ot)

            def emit_wo(qc4, tail=False):
                # output projection for one 512-wide q chunk, overlapping the
                # next chunk's attention. Concurrent row-group matmuls may not
                # accumulate into the same PSUM bank (HW hang): one
                # accumulator per row group, DVE adds them.
                qc = qc4 // 2
                for qt4 in range(4):
                    qt = 4 * qc4 + qt4
                    off = (qt % 8) * P
                    osb = outp.tile([P, D], BF16, tag="osb", name="osb")
                    for dh in range(2):
                        wops0 = psA.tile([P, 512], F32, tag="A", name="wops0", bufs=4)
                        wops1 = psA.tile([P, 512], F32, tag="A", name="wops1", bufs=4)
                        for p in range(2):
                            for a in range(2):
                                lo, hi = SPH * a, SPH * (a + 1)
                                wx = wops0 if a == 0 else wops1
                                nc.tensor.matmul(
                                    wx[:, :],
                                    outT_qc[qc][lo:hi, p, off:off + P],
                                    wo_sb[lo:hi, p, 512 * dh:512 * (dh + 1)],
                                    start=(p == 0), stop=(p == 1))
                        tcp = outp.tile([P, 512], F32, tag="tcp", name="tcp")
                        if tail:
                            nc.scalar.copy(tcp, wops1)
                        else:
                            nc.vector.tensor_copy(tcp, wops1)
                        nc.vector.tensor_add(osb[:, 512 * dh:512 * (dh + 1)],
                                             wops0, tcp)
                    nc.sync.dma_start(out=out_d[P * qt:P * (qt + 1), :], in_=osb)

            def attn_block(qc4, p):
                q0 = 512 * qc4
                poAB = [psA.tile([SPH + 1, 512], F32, tag="A", name=f"po{a}",
                                 bufs=4)
                        for a in range(2)]
                pend = None
                for ct in range(CT + 1):
                    if ct < CT:
                        Sp = psS.tile([P, 2, 512], F32, tag="S", name="Sp",
                                      bufs=2)
                        for a in range(2):
                            lo, hi = SPH * a, SPH * (a + 1)
                            nc.tensor.matmul(
                                Sp[:, a, :],
                                kTp[p][lo:hi, P * ct:P * (ct + 1)],
                                qTp[p][lo:hi, q0:q0 + 512],
                                start=True, stop=True)
                        if masked:
                            mk = ptp.tile([P, 512], BF16, tag="mk", name="mk")
                            nc.sync.dma_start(
                                out=mk,
                                in_=mk_d[P * ct:P * (ct + 1), q0:q0 + 512])
                            for a in range(2):
                                nc.vector.tensor_add(Sp[:, a, :], Sp[:, a, :], mk)
                    if ct >= 1:
                        pct = ct - 1
                        PT = ptp.tile([P, 2, 512], BF16, tag="PT", name="PT")
                        nc.scalar.activation(
                            PT[:, :, :], pend[:, :, :],
                            mybir.ActivationFunctionType.Exp)
                        for a in range(2):
                            nc.tensor.matmul(
                                poAB[a][:, :],
                                vaug[:, pct // 2, 2 * p + a, pct % 2, 0:SPH + 1],
                                PT[:, a, :],
                                start=(pct == 0), stop=(pct == CT - 1))
                    if ct < CT:
                        pend = Sp
                for a in range(2):
                    epilogue(poAB[a], p, a, qc4)

            # attention blocks start as early as possible; projection chunks
            # are spread between blocks so each PE bulge fits the ACT-bound
            # cover of one block. wo lags one q-chunk behind.
            for qc4 in range(4):
                emit_proj_chunk(0, "k", qc4)
            emit_v()
            emit_proj_chunk(0, "q", 0)
            attn_block(0, 0)
            emit_proj_chunk(0, "q", 1)
            for qc4 in range(4):
                emit_proj_chunk(1, "k", qc4)
            emit_proj_chunk(1, "q", 0)
            attn_block(0, 1)
            attn_block(1, 0)
            emit_proj_chunk(1, "q", 1)
            attn_block(1, 1)
            emit_wo(0)
            emit_proj_chunk(0, "q", 2)
            attn_block(2, 0)
            emit_proj_chunk(1, "q", 2)
            attn_block(2, 1)
            emit_wo(1)
            emit_proj_chunk(0, "q", 3)
            attn_block(3, 0)
            emit_proj_chunk(1, "q", 3)
            attn_block(3, 1)
            emit_wo(2)
            emit_wo(3, tail=True)

    nc.compile()
    return nc


def _get_nc(masked: bool):
    if masked not in _NC_CACHE:
        _NC_CACHE[masked] = _build(masked)
    return _NC_CACHE[masked]


def kernel(query, context, attention_mask, Wq, Wk, Wv, Wo, **_unused):
    query = np.asarray(query, dtype=np.float32)
    context = np.asarray(context, dtype=np.float32)
    attention_mask = np.asarray(attention_mask, dtype=np.float32)
    Wq = np.asarray(Wq, dtype=np.float32)
    Wk = np.asarray(Wk, dtype=np.float32)
    Wv = np.asarray(Wv, dtype=np.float32)
    Wo = np.asarray(Wo, dtype=np.float32)

    masked = bool(np.any(attention_mask))
    nc = _get_nc(masked)

    bf = ml_dtypes.bfloat16
    # fold the 1/sqrt(SPH) score scale into Wq
    wq_s = (Wq * (SPH ** -0.5)).astype(bf)
    wk_s = Wk.astype(bf)
    wv_s = Wv.astype(bf)
    wo_s = Wo.astype(bf)

    qtT = [np.ascontiguousarray(query[b].T).astype(bf) for b in range(B)]
    ctT = [np.ascontiguousarray(context[b].T).astype(bf) for b in range(B)]
    if masked:
        mkT = [np.ascontiguousarray((attention_mask[b, 0] * NEG_INF).T).astype(bf)
               for b in range(B)]

    in_maps = []
    for c in range(8):
        b, g = c // 4, c % 4
        hs = slice(NH * g, NH * (g + 1))
        im = {
            "qt": qtT[b],
            "ctx": ctT[b],
            "wq": np.ascontiguousarray(wq_s[:, hs, :]).reshape(D, NH * SPH),
            "wk": np.ascontiguousarray(wk_s[:, hs, :]).reshape(D, NH * SPH),
            "wv": np.ascontiguousarray(wv_s[:, hs, :]).reshape(D, NH * SPH),
            "wo": np.ascontiguousarray(wo_s[hs]).reshape(NH * SPH, D),
        }
        if masked:
            im["maskT"] = mkT[b]
        in_maps.append(im)

    res = run_bass_kernel_spmd(nc, in_maps, core_ids=list(range(8)))

    out = np.zeros((B, S, D), dtype=np.float32)
    for c in range(8):
        out[c // 4] += res.results[c]["out"].astype(np.float32)
    return out



# revision 50
# speedup vs baseline: 1.4742x; 1.4742x over previous
"""Multi-head attention (B=2, S=2048, D=1024, H=16) on 8 Trainium2 NeuronCores.

Sharding: data-parallel on batch, tensor-parallel on heads.
Core c handles batch b = c // 4 and heads [4*(c%4), 4*(c%4)+4).
Each core computes its 4 heads' attention + its partial Wo projection;
the host sums the 4 partial [S, D] outputs per batch (the TP all-reduce).

Device-side layout (all bf16 storage, fp32 PSUM accumulation):
- Host pre-transposes query/context to [D, S] so every matmul contraction
  lands on SBUF partitions with no on-device transposes.
- Scores are computed transposed ([c, q]); the PV matmul consumes the exp'd
  probabilities as the moving operand, and the softmax denominator comes
  free as a 65th row of the PV output (ones column in V).
- Wo is one fused matmul per (q-tile, d-half, pair): contraction 128 covers
  both heads of a pair at once (outT rows 0-63 = head a=0, 64-127 = a=1).
- The attention inner loop is exp-paced (ACT 1038ns/ct vs PE 852ns/ct), so
  projection / Wo / epilogue work is interleaved INTO the loop as "fillers",
  keeping PE saturated while ACT streams exp.
- Softmax normalization: po is staged to SBUF eagerly (frees its PSUM slot
  for the next block), denominator row shifted to partition 0 by DMA,
  gpsimd-broadcast, reciprocal, one DVE multiply.
- Inputs are DMA'd in 512-column chunks ordered so the first k-projection
  piece starts ~5.5us in, racing the remaining input DMA.
- PSUM budget (8 banks): scores 2bufs x [128,2,512] = 4, po pair = 2,
  shared proj/V/wo ring = 2. The tail borrows all 8 banks for wo(3)'s
  split-phase accumulators.
- vaug / outT are split into many small tiles and q-projection fillers run
  in opposite-pair blocks: Tile hazard tracking is per-tile, so shared big
  tiles serialize on false dependencies.
"""

import numpy as np
import ml_dtypes

import concourse.bacc as bacc
import concourse.mybir as mybir
from concourse.tile import TileContext
from concourse.bass_utils import run_bass_kernel_spmd

BF16 = mybir.dt.bfloat16
F32 = mybir.dt.float32

B, S, D, H = 2, 2048, 1024, 16
SPH = D // H          # 64
NH = 4                # heads per core
P = 128               # SBUF partitions
DC = D // P           # 8 d-chunks
CT = S // P           # 16 c-tiles
NEG_INF = -1e9

_NC_CACHE = {}
# debug switch: emit all filler generators between blocks instead of
# interleaved into them (baseline-style scheduling)
NOFILL = False


def _build(masked: bool):
    nc = bacc.Bacc("TRN2", target_bir_lowering=False, debug=False, num_devices=8)

    qt_d = nc.declare_dram_parameter("qt", [D, S], BF16, isOutput=False)
    ct_d = nc.declare_dram_parameter("ctx", [D, S], BF16, isOutput=False)
    wq_d = nc.declare_dram_parameter("wq", [D, NH * SPH], BF16, isOutput=False)
    wk_d = nc.declare_dram_parameter("wk", [D, NH * SPH], BF16, isOutput=False)
    wv_d = nc.declare_dram_parameter("wv", [D, NH * SPH], BF16, isOutput=False)
    wo_d = nc.declare_dram_parameter("wo", [NH * SPH, D], BF16, isOutput=False)
    if masked:
        mk_d = nc.declare_dram_parameter("maskT", [S, S], BF16, isOutput=False)
    out_d = nc.declare_dram_parameter("out", [S, D], BF16, isOutput=True)

    with TileContext(nc) as tc:
        with (
            tc.tile_pool(name="const", bufs=1) as const,
            tc.tile_pool(name="work", bufs=1) as work,
            tc.tile_pool(name="pt", bufs=4) as ptp,
            tc.tile_pool(name="outp", bufs=3) as outp,
            tc.tile_pool(name="epi", bufs=4) as epi,
            tc.tile_pool(name="psS", bufs=2, space="PSUM") as psS,
            tc.tile_pool(name="psA", bufs=2, space="PSUM") as psA,
            tc.tile_pool(name="psW", bufs=2, space="PSUM") as psW,
        ):
            # ---- staged inputs ------------------------------------------
            wq_sb = const.tile([P, DC, NH * SPH], BF16)
            wk_sb = const.tile([P, DC, NH * SPH], BF16)
            wv_sb = const.tile([P, DC, NH * SPH], BF16)
            # wo rows are (h, s); head pair t = h//2 packs two heads into the
            # partition dim (head h%2==0 -> partitions 0-63, ==1 -> 64-127).
            wo_sb = const.tile([P, 2, D], BF16)
            qt_sb = const.tile([P, DC, S], BF16)
            ct_sb = const.tile([P, DC, S], BF16)
            qt_r = qt_d[:, :].rearrange("(c p) q -> p c q", p=P)
            ct_r = ct_d[:, :].rearrange("(c p) q -> p c q", p=P)

            # DMA order tuned so the first k-projection piece starts ~5.5us
            # in and each later dependency lands just ahead of its use.
            nc.sync.dma_start(out=wk_sb, in_=wk_d[:, :].rearrange("(c p) n -> p c n", p=P))
            nc.sync.dma_start(out=ct_sb[:, :, 0:256], in_=ct_r[:, :, 0:256])
            nc.sync.dma_start(out=ct_sb[:, :, 256:512], in_=ct_r[:, :, 256:512])
            nc.sync.dma_start(out=wq_sb, in_=wq_d[:, :].rearrange("(c p) n -> p c n", p=P))
            nc.sync.dma_start(out=qt_sb[:, :, 0:512], in_=qt_r[:, :, 0:512])
            nc.sync.dma_start(out=wv_sb, in_=wv_d[:, :].rearrange("(c p) n -> p c n", p=P))
            for i in range(1, 4):
                nc.sync.dma_start(out=ct_sb[:, :, 512 * i:512 * (i + 1)],
                                  in_=ct_r[:, :, 512 * i:512 * (i + 1)])
            for i in range(1, 4):
                nc.sync.dma_start(out=qt_sb[:, :, 512 * i:512 * (i + 1)],
                                  in_=qt_r[:, :, 512 * i:512 * (i + 1)])
            nc.sync.dma_start(out=wo_sb, in_=wo_d[:, :].rearrange("(t x) d -> x t d", x=P))

            # ---- persistent SBUF tensors --------------------------------
            qTp = [work.tile([P, S], BF16, tag=f"qT{p}", name=f"qT{p}") for p in range(2)]
            kTp = [work.tile([P, S], BF16, tag=f"kT{p}", name=f"kT{p}") for p in range(2)]
            # vaug and outT are split into small tiles (per c-tile / per
            # (q-chunk-pair, pair)): Tile's hazard tracking is per-tile, so
            # shared big tiles create false deps between concurrent writers
            # (epilogue DMAs) and readers (wo/PV matmuls)
            vaug = [work.tile([P, NH, 80], BF16, tag=f"va{ct}", name=f"va{ct}")
                    for ct in range(CT)]
            for ct in range(CT):
                nc.vector.memset(vaug[ct][:, :, SPH:SPH + 1], 1.0)
            outTp = [[work.tile([P, 1024], BF16, tag=f"oT{qc}{p2}",
                                name=f"oT{qc}{p2}") for p2 in range(2)]
                     for qc in range(2)]
            # duplicate of wo's a=1 rows at partition base 0: lets the tail
            # wo read the last epilogue's normalized a=1 tile (`ot`, base 0)
            # directly, keeping the ot -> outT partition-shift DMA off the
            # critical path
            wo_a1 = work.tile([SPH, 2, D], BF16, tag="woa1", name="woa1")
            nc.gpsimd.dma_start(out=wo_a1, in_=wo_sb[SPH:P, :, :])

            # ---- filler generators: each yield = ~2 PE matmuls ----------
            def gen_proj(which, p, qc4, csplit=None):
                """q/k projection chunk -> qTp/kTp[p][:, 512*qc4:...].
                csplit=(lo,hi) restricts to a column sub-range (lead-in)."""
                src_sb = wk_sb if which == "k" else wq_sb
                x_sb = ct_sb if which == "k" else qt_sb
                dst = kTp[p] if which == "k" else qTp[p]
                lo, hi = csplit if csplit else (512 * qc4, 512 * (qc4 + 1))
                w = hi - lo
                ps = psW.tile([P, w], F32, tag="W", name=f"ps{which}{p}{qc4}")
                for dc in range(DC):
                    nc.tensor.matmul(
                        ps[:, :],
                        src_sb[:, dc, P * p:P * (p + 1)],
                        x_sb[:, dc, lo:hi],
                        start=(dc == 0), stop=(dc == DC - 1),
                    )
                    if dc % 2 == 1 and dc != DC - 1:
                        yield
                # copy on DVE, not ACT: an ACT copy interleaved between exp's
                # would delay the exp stream that paces the attention loop
                nc.vector.tensor_copy(dst[:, lo:hi], ps[:, :])
                yield

            def gen_vproj(ct):
                """V projection for one c-tile into vaug (+ ones col)."""
                psv = psW.tile([P, NH * SPH], F32, tag="W", name=f"psv{ct}")
                for dc in range(DC):
                    nc.tensor.matmul(
                        psv[:, :],
                        ct_sb[:, dc, P * ct:P * (ct + 1)],
                        wv_sb[:, dc, :],
                        start=(dc == 0), stop=(dc == DC - 1),
                    )
                nc.vector.tensor_copy(
                    vaug[ct][:, :, 0:SPH],
                    psv[:, :].rearrange("p (h s) -> p h s", h=NH),
                )
                yield

            def gen_wo(qc4):
                """Fused output projection for q-chunk qc4: per (qt, dh) one
                [128,512] accumulator, contraction 128 = both heads of a pair
                per matmul, accumulated over the 2 pairs."""
                qc = qc4 // 2
                for qt4 in range(4):
                    qt = 4 * qc4 + qt4
                    off = (qt % 8) * P
                    osb = outp.tile([P, D], BF16, tag="osb", name="osb")
                    for dh in range(2):
                        wx = psW.tile([P, 512], F32, tag="W", name="wx")
                        for p2 in range(2):
                            nc.tensor.matmul(
                                wx[:, :],
                                outTp[qc][p2][:, off:off + P],
                                wo_sb[:, p2, 512 * dh:512 * (dh + 1)],
                                start=(p2 == 0), stop=(p2 == 1))
                        nc.vector.tensor_copy(osb[:, 512 * dh:512 * (dh + 1)], wx)
                        if dh == 0:
                            yield
                    # DMA emitted before the final yield: a generator pumped an
                    # exact number of steps never runs code after its last yield
                    nc.sync.dma_start(out=out_d[P * qt:P * (qt + 1), :], in_=osb)
                    yield

            # ---- epilogue: normalize po rows 0-63 by row 64 -------------
            # recip of the denominator row in-lane at partition 64 (DVE),
            # broadcast to partitions 0-63 via a 1-row ones matmul (PE),
            # then one DVE multiply straight out of PSUM.
            def gen_epi(poAB, p, qc4):
                qc, off = qc4 // 2, 512 * (qc4 % 2)
                for a in range(2):
                    po = poAB[a]
                    # stage po to SBUF eagerly so the po PSUM slot recycles
                    # before the next block's first PV accumulation
                    staged = epi.tile([SPH + 1, 512], F32, tag="stg", name="stg")
                    nc.vector.tensor_copy(staged, po[:, :])
                    # engines cannot shift partitions: denominator row 64 ->
                    # row 0 via DMA, gpsimd broadcast to rows 0-63, THEN
                    # reciprocal (reciprocal_approx_fast breaks on HW when
                    # run on a 1-row slice at partition base 64)
                    drow = epi.tile([1, 512], F32, tag="drow", name="drow")
                    nc.gpsimd.dma_start(out=drow[0:1, :], in_=staged[SPH:SPH + 1, :])
                    rb = epi.tile([SPH, 512], F32, tag="rb", name="rb")
                    nc.gpsimd.partition_broadcast(rb, drow[0:1, :], channels=SPH)
                    rb2 = epi.tile([SPH, 512], F32, tag="rb2", name="rb2")
                    nc.vector.reciprocal_approx_fast(rb2, rb)
                    if a == 0:
                        nc.vector.tensor_mul(
                            outTp[qc][p][0:SPH, off:off + 512],
                            staged[0:SPH, :], rb2)
                    else:
                        ot = ptp.tile([SPH, 512], BF16, tag="ott", name="ot")
                        nc.vector.tensor_mul(ot, staged[0:SPH, :], rb2)
                        nc.gpsimd.dma_start(
                            out=outTp[qc][p][SPH:P, off:off + 512], in_=ot)
                    yield

            # ---- attention block: two heads of a pair interleaved -------
            # Heads a=0/1 live at partition bases 0/64 in qTp/kTp; scores for
            # both go into one [128,2,512] PSUM tile so a single exp covers
            # both heads. slots: per-ct filler generators (~2 steps each).
            def attn_block(qc4, p, slots):
                q0 = 512 * qc4
                if NOFILL:
                    seen = []
                    for gens in slots.values():
                        for gen, _ in gens:
                            if all(gen is not g for g in seen):
                                seen.append(gen)
                    for gen in seen:
                        for _ in gen:
                            pass
                poAB = [psA.tile([SPH + 1, 512], F32, tag="A", name=f"po{a}")
                        for a in range(2)]
                pend = None
                for ct in range(CT + 1):
                    if ct < CT:
                        Sp = psS.tile([P, 2, 512], F32, tag="S", name="Sp")
                        for a in range(2):
                            lo, hi = SPH * a, SPH * (a + 1)
                            nc.tensor.matmul(
                                Sp[:, a, :],
                                kTp[p][lo:hi, P * ct:P * (ct + 1)],
                                qTp[p][lo:hi, q0:q0 + 512],
                                start=True, stop=True)
                        if masked:
                            mk = ptp.tile([P, 512], BF16, tag="mk", name="mk")
                            nc.sync.dma_start(
                                out=mk,
                                in_=mk_d[P * ct:P * (ct + 1), q0:q0 + 512])
                            for a in range(2):
                                nc.vector.tensor_add(Sp[:, a, :], Sp[:, a, :], mk)
                    if ct >= 1:
                        pct = ct - 1
                        PT = ptp.tile([P, 2, 512], BF16, tag="PT", name="PT")
                        nc.scalar.activation(
                            PT[:, :, :], pend[:, :, :],
                            mybir.ActivationFunctionType.Exp)
                        for a in range(2):
                            nc.tensor.matmul(
                                poAB[a][:, :],
                                vaug[pct][:, 2 * p + a, 0:SPH + 1],
                                PT[:, a, :],
                                start=(pct == 0), stop=(pct == CT - 1))
                    if ct < CT:
                        pend = Sp
                        if not NOFILL:
                            for gen, nsteps in slots.get(ct, ()):
                                for _ in range(nsteps):
                                    try:
                                        next(gen)
                                    except StopIteration:
                                        break
                return poAB

            def drain(gen):
                for _ in gen:
                    pass

            # ---- lead-in: race the input DMA ----------------------------
            drain(gen_proj("k", 0, 0, csplit=(0, 256)))
            drain(gen_proj("k", 0, 0, csplit=(256, 512)))
            drain(gen_proj("k", 1, 0))
            drain(gen_proj("q", 0, 0))

            # block (0,0): V projections + remaining kTp[0] chunks, racing
            # the DMA arrivals (ct chunk i lands ~(11.6+2.9i)us).
            vg = {ct: gen_vproj(ct) for ct in range(CT)}
            g_k01 = gen_proj("k", 0, 1)
            g_k02 = gen_proj("k", 0, 2)
            g_k03 = gen_proj("k", 0, 3)
            g_q10 = gen_proj("q", 1, 0)
            po00 = attn_block(0, 0, {
                0: [(vg[0], 1), (vg[1], 1)], 1: [(vg[2], 1), (vg[3], 1)],
                2: [(g_k01, 2)], 3: [(g_k01, 2)],
                4: [(vg[4], 1), (vg[5], 1)], 5: [(vg[6], 1), (vg[7], 1)],
                6: [(g_k02, 2)], 7: [(g_k02, 2)],
                8: [(vg[8], 1), (vg[9], 1)], 9: [(vg[10], 1), (vg[11], 1)],
                10: [(g_k03, 2)], 11: [(g_k03, 2)],
                12: [(vg[12], 1), (vg[13], 1)], 13: [(vg[14], 1), (vg[15], 1)],
                14: [(g_q10, 2)], 15: [(g_q10, 2)],
            })

            # q-projection fillers are placed in blocks of the OPPOSITE pair
            # (a block's scores read qTp[p]; a same-pair q-proj filler would
            # add a false per-tile hazard on its own score stream)
            g_epi = gen_epi(po00, 0, 0)
            g_k11 = gen_proj("k", 1, 1)
            g_k12 = gen_proj("k", 1, 2)
            g_k13 = gen_proj("k", 1, 3)
            g_q01 = gen_proj("q", 0, 1)
            po01 = attn_block(0, 1, {
                0: [(g_epi, 1), (g_k11, 2)], 1: [(g_epi, 1), (g_k11, 2)],
                2: [(g_k12, 2)], 3: [(g_k12, 2)],
                4: [(g_k13, 2)], 5: [(g_k13, 2)],
                7: [(g_q01, 2)], 9: [(g_q01, 2)],
            })

            g_epi = gen_epi(po01, 1, 0)
            g_wo0 = gen_wo(0)
            g_q11 = gen_proj("q", 1, 1)
            # wo fillers start at iter 7: they read outT rows written by the
            # preceding epilogue's partition-shift DMA (~4.5us into the block)
            po10 = attn_block(1, 0, {
                0: [(g_epi, 1)], 1: [(g_epi, 1)],
                3: [(g_q11, 2)], 5: [(g_q11, 2)],
                7: [(g_wo0, 1)], 8: [(g_wo0, 1)], 9: [(g_wo0, 1)],
                11: [(g_wo0, 1)], 13: [(g_wo0, 1)], 15: [(g_wo0, 1)],
            })

            g_epi = gen_epi(po10, 0, 1)
            g_q02 = gen_proj("q", 0, 2)
            po11 = attn_block(1, 1, {
                0: [(g_epi, 1)], 1: [(g_epi, 1)],
                3: [(g_q02, 2)], 5: [(g_q02, 2)],
                7: [(g_wo0, 1)], 9: [(g_wo0, 1)],
            })

            g_epi = gen_epi(po11, 1, 1)
            g_wo1 = gen_wo(1)
            g_q12 = gen_proj("q", 1, 2)
            po20 = attn_block(2, 0, {
                0: [(g_epi, 1)], 1: [(g_epi, 1)],
                3: [(g_q12, 2)], 5: [(g_q12, 2)],
                7: [(g_wo1, 1)], 8: [(g_wo1, 1)], 9: [(g_wo1, 1)],
                11: [(g_wo1, 1)], 13: [(g_wo1, 1)], 15: [(g_wo1, 1)],
            })

            g_epi = gen_epi(po20, 0, 2)
            g_q03 = gen_proj("q", 0, 3)
            po21 = attn_block(2, 1, {
                0: [(g_epi, 1)], 1: [(g_epi, 1)],
                3: [(g_q03, 2)], 5: [(g_q03, 2)],
                7: [(g_wo1, 1)], 9: [(g_wo1, 1)],
            })

            g_epi = gen_epi(po21, 1, 2)
            g_wo2 = gen_wo(2)
            g_q13 = gen_proj("q", 1, 3)
            # only half of wo(2) here: the rest fills the tail's PE window
            # between the last epilogue's phases (keeps PE ramped)
            po30 = attn_block(3, 0, {
                0: [(g_epi, 1)], 1: [(g_epi, 1)],
                3: [(g_q13, 2)], 5: [(g_q13, 2)],
                7: [(g_wo2, 1)], 9: [(g_wo2, 1)],
            })

            g_epi = gen_epi(po30, 0, 3)
            po31 = attn_block(3, 1, {
                0: [(g_epi, 1)], 1: [(g_epi, 1)],
                7: [(g_wo2, 1)], 9: [(g_wo2, 1)],
            })

            # tail: wo(3) in 3 phases chasing the last epilogue's chain.
            # All other PSUM pools are free by now: 8 accumulators live
            # across psS/psA/psW.
            #   A:  pair-0, full 128-contraction (ready immediately)
            #   B1: pair-1 head a=0 rows (ready after the a=0 multiply)
            #   B2: pair-1 head a=1, read from `ot` at base 0 via wo_a1
            wx8 = []
            sS = [psS.tile([P, 2, 512], F32, tag="S", name=f"twoS{i}")
                  for i in range(2)]
            for i in range(2):
                wx8 += [sS[i][:, 0, :], sS[i][:, 1, :]]
            wx8 += [psA.tile([P, 512], F32, tag="A", name=f"twoA{i}")
                    for i in range(2)]

            def phaseA(g):
                qt4, dh = g // 2, g % 2
                off = ((4 * 3 + qt4) % 8) * P
                nc.tensor.matmul(
                    wx8[g][:, :],
                    outTp[1][0][:, off:off + P],
                    wo_sb[:, 0, 512 * dh:512 * (dh + 1)],
                    start=True, stop=False, skip_group_check=True)

            for g in range(6):
                phaseA(g)
            # leftover wo(2) steps keep PE ramped while the epilogue's
            # broadcast chain runs (psW ring used here, then by wx8[6:])
            drain(g_wo2)
            wx8 += [psW.tile([P, 512], F32, tag="W", name=f"twoW{i}")
                    for i in range(2)]
            for g in range(6, 8):
                phaseA(g)
            # inline last epilogue (pair 1, qc4=3); keeps `ot` for phase B2
            ot31 = None
            for a in range(2):
                po = po31[a]
                staged = epi.tile([SPH + 1, 512], F32, tag="stg", name="stg")
                nc.vector.tensor_copy(staged, po[:, :])
                drow = epi.tile([1, 512], F32, tag="drow", name="drow")
                nc.gpsimd.dma_start(out=drow[0:1, :], in_=staged[SPH:SPH + 1, :])
                rb = epi.tile([SPH, 512], F32, tag="rb", name="rb")
                nc.gpsimd.partition_broadcast(rb, drow[0:1, :], channels=SPH)
                rb2 = epi.tile([SPH, 512], F32, tag="rb2", name="rb2")
                nc.vector.reciprocal_approx_fast(rb2, rb)
                if a == 0:
                    otA = ptp.tile([SPH, 512], BF16, tag="ott", name="otA")
                    nc.vector.tensor_mul(otA, staged[0:SPH, :], rb2)
                else:
                    ot31 = ptp.tile([SPH, 512], BF16, tag="ott", name="ot")
                    nc.vector.tensor_mul(ot31, staged[0:SPH, :], rb2)
            for g in range(8):
                qt4, dh = g // 2, g % 2
                nc.tensor.matmul(
                    wx8[g][:, :],
                    otA[:, P * qt4:P * (qt4 + 1)],
                    wo_sb[0:SPH, 1, 512 * dh:512 * (dh + 1)],
                    start=False, stop=False, skip_group_check=True)
            # final accumulations with copies interleaved and spread across
            # DVE/ACT/Pool so the drain isn't serialized on one engine
            osbt = [outp.tile([P, D], BF16, tag="osb", name=f"osbt{q}")
                    for q in range(3)]
            for g in range(8):
                qt4, dh = g // 2, g % 2
                nc.tensor.matmul(
                    wx8[g][:, :],
                    ot31[:, P * qt4:P * (qt4 + 1)],
                    wo_a1[:, 1, 512 * dh:512 * (dh + 1)],
                    start=False, stop=True, skip_group_check=True)
                osb = osbt[qt4 % 3]
                dst = osb[:, 512 * dh:512 * (dh + 1)]
                if g % 2 == 0:
                    nc.vector.tensor_copy(dst, wx8[g])
                else:
                    nc.scalar.copy(dst, wx8[g])
                if dh == 1:
                    qt = 4 * 3 + qt4
                    nc.sync.dma_start(out=out_d[P * qt:P * (qt + 1), :],
                                      in_=osb)

    nc.compile()
    return nc


def _get_nc(masked: bool):
    if masked not in _NC_CACHE:
        _NC_CACHE[masked] = _build(masked)
    return _NC_CACHE[masked]


def kernel(query, context, attention_mask, Wq, Wk, Wv, Wo, **_unused):
    query = np.asarray(query, dtype=np.float32)
    context = np.asarray(context, dtype=np.float32)
    attention_mask = np.asarray(attention_mask, dtype=np.float32)
    Wq = np.asarray(Wq, dtype=np.float32)
    Wk = np.asarray(Wk, dtype=np.float32)
    Wv = np.asarray(Wv, dtype=np.float32)
    Wo = np.asarray(Wo, dtype=np.float32)

    masked = bool(np.any(attention_mask))
    nc = _get_nc(masked)

    bf = ml_dtypes.bfloat16
    # fold the 1/sqrt(SPH) score scale into Wq
    wq_s = (Wq * (SPH ** -0.5)).astype(bf)
    wk_s = Wk.astype(bf)
    wv_s = Wv.astype(bf)
    wo_s = Wo.astype(bf)

    qtT = [np.ascontiguousarray(query[b].T).astype(bf) for b in range(B)]
    ctT = [np.ascontiguousarray(context[b].T).astype(bf) for b in range(B)]
    if masked:
        mkT = [np.ascontiguousarray((attention_mask[b, 0] * NEG_INF).T).astype(bf)
               for b in range(B)]

    in_maps = []
    for c in range(8):
        b, g = c // 4, c % 4
        hs = slice(NH * g, NH * (g + 1))
        im = {
            "qt": qtT[b],
            "ctx": ctT[b],
            "wq": np.ascontiguousarray(wq_s[:, hs, :]).reshape(D, NH * SPH),
            "wk": np.ascontiguousarray(wk_s[:, hs, :]).reshape(D, NH * SPH),
            "wv": np.ascontiguousarray(wv_s[:, hs, :]).reshape(D, NH * SPH),
            "wo": np.ascontiguousarray(wo_s[hs]).reshape(NH * SPH, D),
        }
        if masked:
            im["maskT"] = mkT[b]
        in_maps.append(im)

    global _last_in_maps
    _last_in_maps = in_maps
    res = run_bass_kernel_spmd(nc, in_maps, core_ids=list(range(8)))

    out = np.zeros((B, S, D), dtype=np.float32)
    for c in range(8):
        out[c // 4] += res.results[c]["out"].astype(np.float32)
    return out
